# revision 59
# baseline (speedup 1.0000x reference)
"""Trainium2 Bass kernel for nn_Allocator2 (dense_cnn), 8 NeuronCores.

Pure data parallelism: batch 64 -> 8 samples per core, weights replicated.

Per-core pipeline (all matmuls bf16 operands, fp32 PSUM accumulation —
validated: min |F3 pre-act| = 0.042 while bf16-induced error <= 0.0035):

  head   : 1x1 convs packed across 8 samples with block-diagonal weights
           K=72->M=48 (T1), 48->32 (T2), 32->16 (T3); y: 24->16, 16->16
  dilated: 52-row shifted tensor S (rows = (shift s in 0..25, ch c in 0..1)),
           one Toeplitz matmul pair M=128+47 -> O [175, 8167]
  F1     : conv2d 25ch,H7 -> 16ch,H6, k=2x6: M=(o,h)=96, K=(ci,hh)=175
           (chunks 128+47), kw via 6 rhs offsets accumulated in PSUM
  F2     : 16ch,H6 -> 8ch,H5: M=40, K=96, 6 offsets
  F3     : 8ch,H5 -> 1ch,H4 + round(sigmoid) == threshold z > -bF3.
           K baked x3 (shift 0,1,2) -> K=120, 2 matmuls (offsets 0,3)
"""

import numpy as np
import ml_dtypes

BF16 = ml_dtypes.bfloat16
E4M3 = ml_dtypes.float8_e4m3fn

B = 64            # global batch
NCORES = 8
BS = B // NCORES  # 8 samples per core
ND = 25
L = 8192          # concat length (4096 + 4096)
LX = 4096
LC = L - ND       # 8167 dilated output length
T1 = LC - 5       # 8162 F1 output length
T2 = T1 - 5       # 8157
T3 = T2 - 5       # 8152
NT = 512          # matmul free-dim tile
DEBUG_Z3 = False  # True: emit raw (z3 - thr) instead of thresholded output


def _bd(blocks):
    """block-diagonal stack of 2D arrays"""
    rs = sum(b.shape[0] for b in blocks)
    cs = sum(b.shape[1] for b in blocks)
    out = np.zeros((rs, cs), np.float32)
    r = c = 0
    for b in blocks:
        out[r:r + b.shape[0], c:c + b.shape[1]] = b
        r += b.shape[0]
        c += b.shape[1]
    return out


def build_weights(inp):
    """Host-side weight prep. Returns dict of np arrays (bf16 weights,
    fp32 biases) shared by all cores."""
    w = {}
    f32 = np.float32

    # ---- head: block-diagonal over BS samples, lhsT layout [K, M] ----
    # out[s*Co+o, t] = sum_c wT[o, c] * x[s*Ci+c, t]
    def head_lhsT(wmat):  # wmat [Co, Ci] -> lhsT [Ci, Co] per sample
        return _bd([wmat.T.astype(f32)] * BS)

    w['hT1'] = head_lhsT(inp['wT1'])   # [72, 48]
    w['hT2'] = head_lhsT(inp['wT2'])   # [48, 32]
    w['hT3'] = head_lhsT(inp['wT3'])   # [32, 16]
    w['hR1'] = head_lhsT(inp['wR1'])   # [24, 16]
    w['hR2'] = head_lhsT(inp['wR2'])   # [16, 16]
    for nm, bb in [('bT1', 'bT1'), ('bT2', 'bT2'), ('bT3', 'bT3'),
                   ('bR1', 'bR1'), ('bR2', 'bR2')]:
        w['h' + nm] = np.tile(inp[bb].astype(f32), BS)[:, None]  # [BS*Co, 1]

    # ---- dilated: lhsT [52, 175], rows r=(c*26+sh), cols m=(i*7+o) ----
    dil = np.zeros((52, 175), f32)
    wM = inp['wM'].astype(f32)  # [25, 7, 2, 2]
    for i in range(ND):
        for o in range(7):
            m = i * 7 + o
            for c in range(2):
                dil[c * 26 + 0, m] = wM[i, o, c, 0]          # shift 0 tap
                dil[c * 26 + (i + 1), m] = wM[i, o, c, 1]    # shift i+1 tap
    w['dilA'] = dil[:, :128]
    w['dilB'] = dil[:, 128:]
    bM = np.zeros((175,), f32)
    for i in range(ND):
        for o in range(7):
            bM[i * 7 + o] = inp['bM'][i, o]
    w['bMA'] = bM[:128, None]
    w['bMB'] = bM[128:, None]

    # ---- F1: lhsT[dw] [175, 96], K rows k=(ci*7+hh), M cols m=(o*6+h) ----
    wF1 = inp['wF1'].astype(f32)  # [16, 25, 2, 6]
    f1 = np.zeros((6, 175, 96), f32)
    for dw in range(6):
        for ci in range(25):
            for hh in range(7):
                for o in range(16):
                    for h in range(6):
                        dh = hh - h
                        if 0 <= dh <= 1:
                            f1[dw, ci * 7 + hh, o * 6 + h] = wF1[o, ci, dh, dw]
    w['F1A'] = f1[:, :128, :]   # [6, 128, 96]
    # B-chunk baked x2: rows r<47 hold (k, p=0, dw=2g); rows 64..110 hold
    # (k-64, p=1, dw=2g+1) where the data row provides O[128+k, t+p].
    # Rows 47-63 are zero (engine writes need partition base 0 or 64).
    f1b = np.zeros((3, 111, 96), f32)
    for g in range(3):
        f1b[g, :47, :] = f1[2 * g, 128:, :]
        f1b[g, 64:, :] = f1[2 * g + 1, 128:, :]
    w['F1B'] = f1b
    w['bF1'] = np.repeat(inp['bF1'].astype(f32), 6)[:, None]  # [96,1] rows o*6+h

    # ---- F2: lhsT[dw] [96, 40], K k=(ci*6+hh), M m=(o*5+h) ----
    wF2 = inp['wF2'].astype(f32)  # [8, 16, 2, 6]
    f2 = np.zeros((6, 96, 40), f32)
    for dw in range(6):
        for ci in range(16):
            for hh in range(6):
                for o in range(8):
                    for h in range(5):
                        dh = hh - h
                        if 0 <= dh <= 1:
                            f2[dw, ci * 6 + hh, o * 5 + h] = wF2[o, ci, dh, dw]
    w['F2'] = f2
    w['bF2'] = np.repeat(inp['bF2'].astype(f32), 5)[:, None]  # [40,1]

    # ---- F3 baked x2 (blocked): lhsT[g] [104, 4]; rows k<40 hold
    # (k, p=0, dw=2g), rows 64..103 hold (k-64, p=1, dw=2g+1); block p
    # provides a2[k, t+p]; matmul g in {0,1,2} uses rhs offset 2g.
    # Rows 40-63 zero (engine writes need partition base 0 or 64).
    wF3 = inp['wF3'].astype(f32)  # [1, 8, 2, 6]
    f3 = np.zeros((3, 104, 4), f32)
    for g in range(3):
        for p in range(2):
            dw = 2 * g + p
            for ci in range(8):
                for hh in range(5):
                    for h in range(4):
                        dh = hh - h
                        if 0 <= dh <= 1:
                            f3[g, p * 64 + ci * 5 + hh, h] = wF3[0, ci, dh, dw]
    w['F3'] = f3
    w['thr'] = np.full((4, 1), -inp['bF3'][0], f32)  # out = (psum > thr)

    # bf16-ify matmul weights
    for k in ('hT1', 'hT2', 'hT3', 'hR1', 'hR2', 'dilA', 'dilB',
              'F1A', 'F1B', 'F2', 'F3'):
        w[k] = w[k].astype(BF16)

    # ---- packed transfers: per-row DMA packets are ~48-160ns each, so
    # loading each small weight separately costs 2000+ ring packets.
    # Column-concatenate everything into 4 pack tensors (one DMA each);
    # the kernel views each weight as a column slice of the pack tile.
    wph = np.zeros((72, 128), BF16)            # head weights
    wph[:72, 0:48] = w['hT1']
    wph[:48, 48:80] = w['hT2']
    wph[:32, 80:96] = w['hT3']
    wph[:24, 96:112] = w['hR1']
    wph[:16, 112:128] = w['hR2']
    bph = np.zeros((48, 5), np.float32)        # head biases (col each)
    bph[:48, 0] = w['hbT1'][:, 0]
    bph[:32, 1] = w['hbT2'][:, 0]
    bph[:16, 2] = w['hbT3'][:, 0]
    bph[:16, 3] = w['hbR1'][:, 0]
    bph[:16, 4] = w['hbR2'][:, 0]
    wpm = np.zeros((128, 1291), BF16)          # main weights
    wpm[:52, 0:128] = w['dilA']
    wpm[:52, 128:175] = w['dilB']
    for dw in range(6):
        wpm[:128, 175 + 96 * dw:175 + 96 * (dw + 1)] = w['F1A'][dw]
    for g in range(3):
        wpm[:111, 751 + 96 * g:751 + 96 * (g + 1)] = w['F1B'][g]
    for dw in range(6):
        wpm[:96, 1039 + 40 * dw:1039 + 40 * (dw + 1)] = w['F2'][dw]
    for g in range(3):
        wpm[:104, 1279 + 4 * g:1279 + 4 * (g + 1)] = w['F3'][g]
    bpm = np.zeros((128, 5), np.float32)       # main biases
    bpm[:128, 0] = w['bMA'][:, 0]
    bpm[:47, 1] = w['bMB'][:, 0]
    bpm[:96, 2] = w['bF1'][:, 0]
    bpm[:40, 3] = w['bF2'][:, 0]
    bpm[:4, 4] = w['thr'][:, 0]
    w['wph'], w['bph'], w['wpm'], w['bpm'] = wph, bph, wpm, bpm

    # ---- F1 in fp8 DoubleRow: slot pairs (2i, 2i+1) hold the dw=2i /
    # dw=2i+1 lhsT planes; slots 6-7 = F1B[0..1] DR pair; slot 8 = F1B[2]
    wpf8 = np.zeros((128, 10, 96), E4M3)
    for dw in range(6):
        wpf8[:128, dw, :] = f1[dw, :128, :].astype(E4M3)
    for g in range(3):
        wpf8[:111, 6 + g, :] = f1b[g].astype(E4M3)
    w['wpf8'] = wpf8
    # dilated also in fp8 (margin-validated): S halves its DMA bytes
    wpd8 = np.zeros((52, 175), E4M3)
    wpd8[:, :128] = dil[:, :128].astype(E4M3)
    wpd8[:, 128:] = dil[:, 128:].astype(E4M3)
    w['wpd8'] = wpd8
    # F2 in fp8 DoubleRow: slot pairs (2i, 2i+1) = dw planes
    wpq8 = np.zeros((96, 6, 48), E4M3)   # 48-wide slots: DR plane stride
    for dw in range(6):                   # must be 16B-aligned (40 is not)
        wpq8[:, dw, :40] = f2[dw].astype(E4M3)
    w['wpq8'] = wpq8
    return w


def emulate_core(w, x_core, y_core):
    """Numpy emulation of exactly what the Bass kernel computes for one
    core. x_core [72, 4096] bf16, y_core [24, 4096] bf16. Returns
    [BS, 4, T3] f32 in {0,1}."""
    f32 = np.float32

    def mm(lhsT, rhs):  # bf16 operands, f32 accumulate
        return lhsT.astype(f32).T @ rhs.astype(f32)

    relu = lambda a: np.maximum(a, 0)
    sig = lambda a: 1.0 / (1.0 + np.exp(-a))

    a = relu(mm(w['hT1'], x_core) + w['hbT1']).astype(BF16)
    a = relu(mm(w['hT2'], a) + w['hbT2']).astype(BF16)
    t3 = (mm(w['hT3'], a) + w['hbT3']).astype(E4M3)          # [16, 4096]
    b_ = relu(mm(w['hR1'], y_core) + w['hbR1']).astype(BF16)
    b_ = relu(mm(w['hR2'], b_) + w['hbR2']).astype(E4M3)     # [16, 4096]
    out2 = np.concatenate([t3, b_], axis=1)                  # [16, 8192] fp8

    res = np.zeros((BS, 4, T3), f32)
    for s in range(BS):
        o2 = out2[s * 2:s * 2 + 2]                           # [2, 8192]
        S = np.zeros((52, LC), E4M3)
        for c in range(2):
            for sh in range(26):
                S[c * 26 + sh] = o2[c, sh:sh + LC]
        # dilated + F1 run in fp8 (e4m3 operands, fp32 accumulate)
        Oa = relu(mm(w['wpd8'][:, :128], S) + w['bMA']).astype(E4M3)
        Obp = relu(mm(w['wpd8'][:, 128:], S) + w['bMB']).astype(E4M3)
        Ob = np.zeros((111, LC), E4M3)
        Ob[:47] = Obp
        Ob[64:, :LC - 1] = Obp[:, 1:]
        z1 = np.zeros((96, T1), f32)
        for dw in range(6):
            z1 += mm(w['wpf8'][:, dw, :], Oa[:, dw:dw + T1])
        for g in range(3):
            z1 += mm(w['wpf8'][:111, 6 + g, :], Ob[:, 2 * g:2 * g + T1])
        a1 = sig(z1 + w['bF1']).astype(E4M3)                 # [96, T1]
        z2 = np.zeros((40, T2), f32)
        for dw in range(6):
            z2 += mm(w['wpq8'][:, dw, :40], a1[:, dw:dw + T2])
        a2 = sig(z2 + w['bF2']).astype(BF16)                 # [40, T2]
        a2b = np.zeros((104, T2), BF16)
        a2b[:40] = a2
        a2b[64:, :T2 - 1] = a2[:, 1:]
        z3 = (mm(w['F3'][0], a2b[:, :T3]) + mm(w['F3'][1], a2b[:, 2:2 + T3])
              + mm(w['F3'][2], a2b[:, 4:4 + T3]))
        res[s] = (z3 > w['thr']).astype(f32)                 # [4, T3]
    return res


def _shard_inputs(inputs):
    """Build per-core in_maps (host-side prep + shard)."""
    w = build_weights(inputs)
    in_maps = []
    for c in range(NCORES):
        m = dict(w)
        xs = inputs['x'][c * BS:(c + 1) * BS]  # [8, 9, 4096]
        ys = inputs['y'][c * BS:(c + 1) * BS]
        m['x'] = np.ascontiguousarray(xs.reshape(BS * 9, LX)).astype(BF16)
        m['y'] = np.ascontiguousarray(ys.reshape(BS * 3, LX)).astype(BF16)
        in_maps.append(m)
    return in_maps


# ---------------------------------------------------------------------------
# Bass program
# ---------------------------------------------------------------------------

def _split_excess_waits(bir, maxw=1):
    """The walrus build in this container refuses instructions carrying
    more than ~1 semaphore wait ("Too many sync wait commands").  Tile
    attaches multi-waits freely.  Splitting is semantics-preserving: move
    excess waits onto injected NoOps on the same engine immediately
    before the instruction (engines execute their instruction stream in
    order, so wait-all is preserved)."""
    for fn in bir['functions']:
        for bb in fn['blocks']:
            out = []
            for inst in bb['instructions']:
                si = inst.get('sync_info')
                waits = (si or {}).get('on_wait') or []
                if len(waits) > maxw:
                    extra, keep = waits[:-maxw], waits[-maxw:]
                    for i in range(0, len(extra), maxw):
                        out.append({
                            "debug": inst.get("debug", 0),
                            "engine": inst["engine"], "ins": [],
                            "name": f"{inst['name']}-wsplit{i}",
                            "opcode": "NoOp", "outs": [],
                            "sync_info": {"on_update": [],
                                          "on_wait": extra[i:i + maxw]}})
                    si['on_wait'] = keep
                out.append(inst)
            bb['instructions'] = out
    return bir


def _patch_serialization(nc):
    import orjson
    bir = _split_excess_waits(nc.to_json())
    patched = orjson.dumps(bir)
    nc.to_json_bytes = lambda: patched
    return nc


def build_bass():
    import bass_rust
    import concourse.bass as bass
    import concourse.mybir as mybir
    from concourse.tile import TileContext

    dt = mybir.dt
    AF = mybir.ActivationFunctionType
    ALU = mybir.AluOpType
    MPM = mybir.MatmulPerfMode

    nc = bass.Bass()

    p = {}
    p['x'] = nc.declare_dram_parameter('x', [BS * 9, LX], dt.bfloat16, False)
    p['y'] = nc.declare_dram_parameter('y', [BS * 3, LX], dt.bfloat16, False)
    p['wph'] = nc.declare_dram_parameter('wph', [72, 128], dt.bfloat16, False)
    p['bph'] = nc.declare_dram_parameter('bph', [48, 5], dt.float32, False)
    p['wpm'] = nc.declare_dram_parameter('wpm', [128, 1291], dt.bfloat16,
                                         False)
    p['bpm'] = nc.declare_dram_parameter('bpm', [128, 5], dt.float32, False)
    p['wpf8'] = nc.declare_dram_parameter('wpf8', [128, 10, 96], dt.float8e4,
                                          False)
    p['wpd8'] = nc.declare_dram_parameter('wpd8', [52, 175], dt.float8e4,
                                          False)
    p['wpq8'] = nc.declare_dram_parameter('wpq8', [96, 6, 48], dt.float8e4,
                                          False)
    out_d = nc.declare_dram_parameter('out', [BS * 4, T3], dt.bfloat16, True)

    def ceil_div(a, b):
        return -(-a // b)

    G = 4  # weight-stationary group: reuse each loaded weight across G tiles

    with TileContext(nc) as tc:
        with tc.tile_pool(name="wpool", bufs=1) as wp, \
             tc.tile_pool(name="head", bufs=1) as hp, \
             tc.tile_pool(name="big", bufs=2) as bp, \
             tc.tile_pool(name="psum", bufs=8, space="PSUM") as pp:

            # packed weight loads: one DMA per pack (few fat packets)
            wpht = wp.tile([72, 128], dt.bfloat16, name="wpht")
            bpht = wp.tile([48, 5], dt.float32, name="bpht")
            wpmt = wp.tile([128, 1291], dt.bfloat16, name="wpmt")
            bpmt = wp.tile([128, 5], dt.float32, name="bpmt")
            # scalar-ring DMAs (DMA_DIRECT2D) block the ACT engine for the
            # whole transfer -> only sync (idle SP engine) + gpsimd
            # (async software DGE) carry bulk data
            wpf8t = wp.tile([128, 10, 96], dt.float8e4, name="wpf8t")
            wpd8t = wp.tile([52, 175], dt.float8e4, name="wpd8t")
            wpq8t = wp.tile([96, 6, 48], dt.float8e4, name="wpq8t")
            nc.sync.dma_start(out=wpd8t[...], in_=p['wpd8'][...])
            nc.sync.dma_start(out=wpq8t[...], in_=p['wpq8'][...])
            nc.sync.dma_start(out=wpht[...], in_=p['wph'][...])
            nc.sync.dma_start(out=bpht[...], in_=p['bph'][...])
            nc.gpsimd.dma_start(out=bpmt[...], in_=p['bpm'][...])
            nc.gpsimd.dma_start(out=wpf8t[...], in_=p['wpf8'][...])

            W = {
                'hT1': wpht[:72, 0:48], 'hT2': wpht[:48, 48:80],
                'hT3': wpht[:32, 80:96], 'hR1': wpht[:24, 96:112],
                'hR2': wpht[:16, 112:128],
                'hbT1': bpht[:48, 0:1], 'hbT2': bpht[:32, 1:2],
                'hbT3': bpht[:16, 2:3], 'hbR1': bpht[:16, 3:4],
                'hbR2': bpht[:16, 4:5],
                'dilA': wpd8t[:52, 0:128], 'dilB': wpd8t[:52, 128:175],
                'F1A': [wpmt[:128, 175 + 96 * dw:175 + 96 * (dw + 1)]
                        for dw in range(6)],
                'F1B': [wpmt[:111, 751 + 96 * g:751 + 96 * (g + 1)]
                        for g in range(3)],
                'F2': [wpmt[:96, 1039 + 40 * dw:1039 + 40 * (dw + 1)]
                       for dw in range(6)],
                'F3': [wpmt[:104, 1279 + 4 * g:1279 + 4 * (g + 1)]
                       for g in range(3)],
                'bMA': bpmt[:128, 0:1], 'bMB': bpmt[:47, 1:2],
                'bF1': bpmt[:96, 2:3], 'bF2': bpmt[:40, 3:4],
                'thr': bpmt[:4, 4:5],
                'F1AD': [wpf8t[:, 2 * i:2 * i + 2, :] for i in range(3)],
                'F1BD': wpf8t[:111, 6:8, :],
                'F1B2': wpf8t[:111, 8, :],
                'F2D': [wpq8t[:, 2 * i:2 * i + 2, :] for i in range(3)],
            }

            # ---------------- head inputs: fast clean-2D sync transfers ----
            xt = hp.tile([BS * 9, LX], dt.bfloat16, name="xt")
            yt = hp.tile([BS * 3, LX], dt.bfloat16, name="yt")
            nc.sync.dma_start(out=xt[...], in_=p['x'][...])
            nc.sync.dma_start(out=yt[...], in_=p['y'][...])
            nc.sync.dma_start(out=wpmt[...], in_=p['wpm'][...])

            o2t = hp.tile([BS * 2, L], dt.float8e4, name="o2t")

            def head_layer(w_nm, b_nm, rows_in, rows_out, src, dst, act,
                           dst_off=0):
                # head is activation-op bound: alternate tiles between the
                # scalar and vector engines so the act chain halves
                for j in range(LX // NT):
                    sl = slice(j * NT, (j + 1) * NT)
                    sl2 = slice(dst_off + j * NT, dst_off + (j + 1) * NT)
                    ps = pp.tile([128, NT], dt.float32, tag="ps")
                    nc.tensor.matmul(ps[:rows_out], W[w_nm],
                                     src[:rows_in, sl], start=True, stop=True)
                    if act == 'relu' and j % 2 == 0:
                        nc.scalar.activation(dst[:rows_out, sl2],
                                             ps[:rows_out], AF.Relu,
                                             bias=W[b_nm])
                    elif act == 'relu':
                        nc.vector.tensor_scalar(dst[:rows_out, sl2],
                                                ps[:rows_out],
                                                W[b_nm], 0.0,
                                                ALU.add, ALU.max)
                    elif j % 2 == 0:
                        nc.scalar.activation(dst[:rows_out, sl2],
                                             ps[:rows_out], AF.Identity,
                                             bias=W[b_nm])
                    else:
                        nc.vector.tensor_scalar(dst[:rows_out, sl2],
                                                ps[:rows_out],
                                                W[b_nm], None, ALU.add)

            a1h = hp.tile([BS * 6, LX], dt.bfloat16, tag="htmp", bufs=2)
            head_layer('hT1', 'hbT1', BS * 9, BS * 6, xt, a1h, 'relu')
            a2h = hp.tile([BS * 6, LX], dt.bfloat16, tag="htmp", bufs=2)
            head_layer('hT2', 'hbT2', BS * 6, BS * 4, a1h, a2h, 'relu')
            head_layer('hT3', 'hbT3', BS * 4, BS * 2, a2h, o2t, 'add')
            b1h = hp.tile([BS * 6, LX], dt.bfloat16, tag="htmp", bufs=2)
            head_layer('hR1', 'hbR1', BS * 3, BS * 2, yt, b1h, 'relu')
            head_layer('hR2', 'hbR2', BS * 2, BS * 2, b1h, o2t, 'relu',
                       dst_off=LX)

            # ---------------- S prefetch: all samples, 3 rings ----------
            # S [52, LC]: row (c*26+sh) = out2[c, sh:sh+LC]; one
            # overlapping-window DMA per (channel, half), pre-issued for
            # every sample (bufs=4 gates reuse via semaphores).  Samples
            # 0/1 are on the critical path: their chunks spread over all
            # rings; later samples prefetch far ahead on one ring each.
            half = 4071  # windows [0,half) only need the T-path cols
            Sts = []
            for s in range(BS):
                St = bp.tile([52, LC], dt.float8e4, tag="S", bufs=4,
                             name=f"St{s}")
                # sample 0 is on the critical path: quarter-chunk so the
                # dilated matmuls can start on partial S
                if s == 0:
                    bounds = (0, 2080, half, 6144, LC)
                else:
                    bounds = (0, half, LC)
                for h0, h1 in zip(bounds[:-1], bounds[1:]):
                    for c in range(2):
                        win = o2t[s * 2 + c:s * 2 + c + 1, h0:h1].copy()
                        win.ap = bass_rust.VecI64Pair(
                            [[L, 1], [1, 26], [1, h1 - h0]])
                        nc.gpsimd.dma_start(
                            out=St[c * 26:(c + 1) * 26, h0:h1], in_=win)
                Sts.append(St)

            # hoisted big tiles (bufs=1 anyway); rows never written by the
            # pipeline are zeroed ONCE so the zero-weight lhsT rows can't
            # multiply leftover NaN garbage into the accumulation.
            # Oa/Ob are fp8 two-plane tensors for the DoubleRow F1:
            # plane 1 = plane 0 shifted left by 1 (Oa) / 2 (Ob) columns.
            LCp = 8176  # plane byte stride (16-aligned fp8 cols)
            Oa = bp.tile([128, 2, LCp], dt.float8e4, name="Oa_t", bufs=1)
            Ob = bp.tile([111, 2, LCp], dt.float8e4, name="Ob_t", bufs=1)
            a1t = bp.tile([96, 2, LCp], dt.float8e4, name="a1_t", bufs=1)
            a2b = bp.tile([104, T2], dt.bfloat16, name="a2b_t", bufs=1)
            # (partition base must be 0 or 64: zero the whole lower half
            # once; live rows are overwritten by the pipeline)
            nc.vector.memset(Ob[:64, :, :], 0.0)
            nc.vector.memset(a2b[:64, :], 0.0)

            # ---------------- per-sample pipeline ----------------
            for s in range(BS):
                St = Sts[s]

                # dilated -> Oa [128, LC], Ob [111, LC] (rows 64+: shift-by-1)
                ntile = ceil_div(LC, NT)
                for jg in range(0, ntile, G):
                    js = range(jg, min(jg + G, ntile))
                    pss = {j: pp.tile([128, NT], dt.float32, tag="ps", name="ps")
                           for j in js}
                    for j in js:
                        t0 = j * NT
                        nt = min(NT, LC - t0)
                        nc.tensor.matmul(pss[j][:, :nt], W['dilA'],
                                         St[:, t0:t0 + nt],
                                         start=True, stop=False)
                    for j in js:
                        t0 = j * NT
                        nt = min(NT, LC - t0)
                        nc.tensor.matmul(pss[j][:47, :nt], W['dilB'],
                                         St[:, t0:t0 + nt],
                                         start=False, stop=True)
                    for j in js:
                        t0 = j * NT
                        nt = min(NT, LC - t0)
                        nc.scalar.activation(Oa[:, 0, t0:t0 + nt],
                                             pss[j][:, :nt],
                                             AF.Relu, bias=W['bMA'])
                        # plane 1 = shift-by-1 copy (scalar; gpsimd must
                        # stay free to generate the S-window descriptors)
                        d0 = max(t0 - 1, 0)
                        nc.scalar.activation(Oa[:, 1, d0:t0 + nt - 1],
                                             Oa[:, 0, d0 + 1:t0 + nt],
                                             AF.Copy)
                        nc.vector.tensor_scalar(Ob[:47, 0, t0:t0 + nt],
                                                pss[j][:47, :nt],
                                                W['bMB'], 0.0,
                                                ALU.add, ALU.max)
                        # baked rows 64-110 = shift-by-1 copy
                        nc.vector.tensor_copy(Ob[64:, 0, d0:t0 + nt - 1],
                                              Ob[:47, 0, d0 + 1:t0 + nt])
                        # plane 1 = full-height shift-by-2 of plane 0
                        d2 = max(t0 - 3, 0)
                        nc.vector.tensor_copy(Ob[:, 1, d2:t0 + nt - 3],
                                              Ob[:, 0, d2 + 2:t0 + nt - 1])

                # F1 fp8 DoubleRow: 3 DR on Oa pairs (dw 2i/2i+1), 1 DR on
                # Ob bake pair (F1B 0/1), 1 normal fp8 (F1B[2] at offset 4)
                n1 = ceil_div(T1, NT)
                for jg in range(0, n1, G):
                    js = range(jg, min(jg + G, n1))
                    pss = {j: pp.tile([128, NT], dt.float32, tag="ps", name="ps")
                           for j in js}
                    for i in range(3):
                        for j in js:
                            t0 = j * NT
                            nt = min(NT, T1 - t0)
                            nc.tensor.matmul(
                                pss[j][:96, :nt], W['F1AD'][i],
                                Oa[:, :, t0 + 2 * i:t0 + 2 * i + nt],
                                start=(i == 0), stop=False,
                                perf_mode=MPM.DoubleRow)
                    for j in js:
                        t0 = j * NT
                        nt = min(NT, T1 - t0)
                        nc.tensor.matmul(pss[j][:96, :nt], W['F1BD'],
                                         Ob[:, :, t0:t0 + nt],
                                         start=False, stop=False,
                                         perf_mode=MPM.DoubleRow)
                    for j in js:
                        t0 = j * NT
                        nt = min(NT, T1 - t0)
                        nc.tensor.matmul(pss[j][:96, :nt], W['F1B2'],
                                         Ob[:111, 0, t0 + 4:t0 + 4 + nt],
                                         start=False, stop=True)
                    for j in js:
                        t0 = j * NT
                        nt = min(NT, T1 - t0)
                        nc.scalar.activation(a1t[:, 0, t0:t0 + nt],
                                             pss[j][:96, :nt],
                                             AF.Sigmoid, bias=W['bF1'])
                        d0 = max(t0 - 1, 0)
                        nc.vector.tensor_copy(a1t[:, 1, d0:t0 + nt - 1],
                                              a1t[:, 0, d0 + 1:t0 + nt])

                # F2, weight-stationary; sigmoid into a2b block 0 (rows
                # 0-39, shift 0); block 1 (rows 64-103, shift 1) is a
                # cheap bf16 SBUF->SBUF copy on the vector engine
                n2 = ceil_div(T2, NT)
                for jg in range(0, n2, G):
                    js = range(jg, min(jg + G, n2))
                    pss = {j: pp.tile([128, NT], dt.float32, tag="ps", name="ps")
                           for j in js}
                    for i in range(3):
                        for j in js:
                            t0 = j * NT
                            nt = min(NT, T2 - t0)
                            nc.tensor.matmul(
                                pss[j][:48, :nt], W['F2D'][i],
                                a1t[:, :, t0 + 2 * i:t0 + 2 * i + nt],
                                start=(i == 0), stop=(i == 2),
                                perf_mode=MPM.DoubleRow)
                    for j in js:
                        t0 = j * NT
                        nt = min(NT, T2 - t0)
                        nc.scalar.activation(a2b[0:40, t0:t0 + nt],
                                             pss[j][:40, :nt],
                                             AF.Sigmoid, bias=W['bF2'])
                        d0 = max(t0 - 1, 0)
                        nc.vector.tensor_copy(a2b[64:, d0:t0 + nt - 1],
                                              a2b[0:40, d0 + 1:t0 + nt])

                # F3 2-baked: 3 weights (rhs offsets 0/2/4); threshold; store
                ot = bp.tile([4, T3], dt.bfloat16, tag="ot", bufs=1)
                n3 = ceil_div(T3, NT)
                for jg in range(0, n3, G):
                    js = range(jg, min(jg + G, n3))
                    pss = {j: pp.tile([128, NT], dt.float32, tag="ps", name="ps")
                           for j in js}
                    for g in range(3):
                        for j in js:
                            t0 = j * NT
                            nt = min(NT, T3 - t0)
                            nc.tensor.matmul(pss[j][:4, :nt], W['F3'][g],
                                             a2b[:, t0 + 2 * g:t0 + 2 * g + nt],
                                             start=(g == 0), stop=(g == 2))
                    for j in js:
                        t0 = j * NT
                        nt = min(NT, T3 - t0)
                        if DEBUG_Z3:
                            nc.vector.tensor_scalar(ot[:, t0:t0 + nt],
                                                    pss[j][:4, :nt],
                                                    W['thr'], None,
                                                    ALU.subtract)
                        else:
                            nc.vector.tensor_scalar(ot[:, t0:t0 + nt],
                                                    pss[j][:4, :nt],
                                                    W['thr'], None,
                                                    ALU.is_gt)
                nc.sync.dma_start(out=out_d[s * 4:(s + 1) * 4, :], in_=ot[...])

    return _patch_serialization(nc)


def kernel(**inputs):
    inputs = {k: np.asarray(v) for k, v in inputs.items()}
    in_maps = _shard_inputs(inputs)
    nc = build_bass()
    from concourse.bass_utils import run_bass_kernel_spmd
    res = run_bass_kernel_spmd(nc, in_maps, core_ids=list(range(NCORES)))
    outs = [res.results[i]['out'].reshape(BS, 4, T3) for i in range(NCORES)]
    full = np.concatenate(outs, axis=0)[:, None]  # [64, 1, 4, T3]
    return full.astype(np.float32)

